# revision 3
# baseline (speedup 1.0000x reference)
"""Trainium2 Bass kernel for nn_ARDecoderECD (GRU->LSTM AR decoder), v3.

v3 vs v2: antiphase software-pipelined emission. The two batch chains are
explicitly interleaved half-a-step apart, with every engine's program order
arranged so the in-order queues never head-of-line block the other chain:

  GRU iteration t:
    [recur mms c0,t][sig_r c0][t1 c0][sig_z c0]      (c0 front)
    [tanh c1,t-1][oz/nz/add c1]                      (c1 back)
    [input mms c1,t][ident-mm c0,t]                  (fillers)
    [recur mms c1,t][sig_r c1][t1 c1][sig_z c1]      (c1 front)
    [tanh c0,t][oz/nz/add c0]                        (c0 back)
    [input mms c0,t+1][ident-mm c1,t]

  LSTM ACT order per iteration:
    tanh_c(c1,t-1) | sig_ifo(c0,t) | g(c0,t) | sig_ifo(c1,t) | g(c1,t)
    | tanh_c(c0,t)

Other v2 features kept: identity-matmul accumulate for the GRU t2 add,
merged i/f/o sigmoid, host-side output projection (kernel emits bf16 h).
oz/m2 on DVE (Pool elementwise costs ~1.1us/op), zh on Pool.
"""

import numpy as np
from contextlib import ExitStack

import concourse.bacc as bacc
import concourse.bass as bass
import concourse.tile as tile
from concourse import mybir
from concourse.bass_utils import run_bass_kernel_spmd

B, T = 8192, 26
V, E, H, L = 23, 100, 128, 64
N_CORES = 8
BC = B // N_CORES  # 1024 samples per core
F32 = mybir.dt.float32
F32R = mybir.dt.float32r
BF16 = mybir.dt.bfloat16
AF = mybir.ActivationFunctionType
ALU = mybir.AluOpType
NCH = 2
CW = BC // NCH  # 512


def _emit(nc, tc, d, ctx, reps=1):
    wp = ctx.enter_context(tc.tile_pool(name="weights", bufs=1))
    run = ctx.enter_context(tc.tile_pool(name="run", bufs=2))
    opool = ctx.enter_context(tc.tile_pool(name="opool", bufs=4))
    def mm(out, lhsT, rhs, start, stop):
        nc.tensor.matmul(out, lhsT, rhs, start=start, stop=stop)

    def wload(name, shape, dt_):
        t = wp.tile(shape, dt_, name=name)
        nc.sync.dma_start(t[:], d[name][:])
        return t

    xg_sb = wload("xg_tab", [V, 3 * H], BF16)
    whh_sb = wload("whh_T", [H, 3 * H], BF16)
    fcw_sb = wload("fcw_T", [L, H], F32R)
    fcb_sb = wload("fc_b", [H, 1], F32)
    bhhn_sb = wload("bhh_n", [H, 1], F32)
    wihl_sb = wload("wih_lT", [H, 4 * E], BF16)
    whhl_sb = wload("whh_laug", [E + 1, 4 * E], BF16)
    ident_sb = wload("ident", [H, H], BF16)
    outw_sb = wload("out_waug", [E + 1, V], BF16)
    zT_sb = wload("zT", [L, BC], F32R)

    chs = [slice(c * CW, (c + 1) * CW) for c in range(NCH)]

    y = [wp.tile([H, T * CW], BF16, name=f"y{c}") for c in range(NCH)]

    hl_t = [[None, None] for _ in range(NCH)]
    for c in range(NCH):
        for idx in range(2):
            hlx = wp.tile([E + 1, CW], BF16, name=f"hl{c}_{idx}")
            nc.sync.dma_start(hlx[:], d["hl_init"][:, 0:CW])
            hl_t[c][idx] = hlx

    O_tiles = {}
    for rep in range(reps):
        h0 = [None] * NCH
        c_prev = [None] * NCH
        g_state = [None] * NCH
        g_prev = [None] * NCH
        l_state = [None] * NCH

        def load_O(t):
            Ot = opool.tile([V, BC], BF16, tag="O", name=f"O{t}_{rep}")
            nc.sync.dma_start(Ot[:], d["O"][t])
            O_tiles[t] = Ot

        def hprev(c, t):
            return h0[c][:] if t == 0 else y[c][:, (t - 1) * CW : t * CW]

        # ================= GRU phase (own PSUM pool) =================
        # Per-bank PSUM tiles: dependency tracking is tile-granular, so
        # separate tiles per gate kill all false intra-tile WAR/RAW hops.
        with tc.tile_pool(name=f"psumG{rep}", bufs=1, space="PSUM") as ppg:
            pr = [ppg.tile([128, CW], F32, name=f"pr{c}_{rep}")
                  for c in range(NCH)]
            pz = [ppg.tile([128, CW], F32, name=f"pz{c}_{rep}")
                  for c in range(NCH)]
            pxn = [ppg.tile([128, CW], F32, name=f"pxn{c}_{rep}")
                   for c in range(NCH)]
            phn = [ppg.tile([128, CW], F32, name=f"phn{c}_{rep}")
                   for c in range(NCH)]

            def emit_h0(c):
                mm(pr[c][:], fcw_sb[:], zT_sb[:, chs[c]],
                   start=True, stop=True)
                hc = run.tile([H, CW], BF16, tag=f"h{c}",
                              name=f"h_init_{rep}_{c}")
                nc.scalar.activation(hc[:], pr[c][:], AF.Tanh,
                                     bias=fcb_sb[:, 0:1])
                h0[c] = hc

            def g_in_mms(t, c):
                Ot = O_tiles[t]
                mm(pr[c][:], xg_sb[:, 0:H], Ot[:, chs[c]],
                   start=True, stop=False)
                mm(pz[c][:], xg_sb[:, H : 2 * H], Ot[:, chs[c]],
                   start=True, stop=False)
                mm(pxn[c][:], xg_sb[:, 2 * H : 3 * H], Ot[:, chs[c]],
                   start=True, stop=False)

            def g_rec_mms(t, c):
                h_prev = hprev(c, t)
                mm(pr[c][:], whh_sb[:, 0:H], h_prev, start=False, stop=True)
                mm(phn[c][:], whh_sb[:, 2 * H : 3 * H], h_prev,
                   start=True, stop=True)
                mm(pz[c][:], whh_sb[:, H : 2 * H], h_prev,
                   start=False, stop=True)

            def g_front(t, c):
                h_prev = hprev(c, t)
                r_sb = run.tile([H, CW], BF16, tag=f"r{c}",
                                name=f"r{t}_{c}_{rep}")
                nc.scalar.activation(r_sb[:], pr[c][:], AF.Sigmoid)
                t1_sb = run.tile([H, CW], BF16, tag=f"t1{c}",
                                 name=f"t1{t}_{c}_{rep}")
                nc.vector.scalar_tensor_tensor(
                    t1_sb[:], phn[c][:], bhhn_sb[:, 0:1], r_sb[:],
                    ALU.add, ALU.mult)
                z_sb = run.tile([H, CW], BF16, tag=f"z{c}",
                                name=f"z{t}_{c}_{rep}")
                nc.scalar.activation(z_sb[:], pz[c][:], AF.Sigmoid)
                oz_sb = run.tile([H, CW], BF16, tag=f"oz{c}",
                                 name=f"oz{t}_{c}_{rep}")
                nc.vector.tensor_scalar(oz_sb[:], z_sb[:], -1.0, 1.0,
                                        ALU.mult, ALU.add)
                zh_sb = run.tile([H, CW], BF16, tag=f"zh{c}",
                                 name=f"zh{t}_{c}_{rep}")
                nc.gpsimd.tensor_mul(zh_sb[:], z_sb[:], h_prev)
                g_state[c] = (t1_sb, oz_sb, zh_sb)

            def g_ident(t, c):
                t1_sb, _, _ = g_state[c]
                mm(pxn[c][:], ident_sb[:], t1_sb[:], start=False, stop=True)

            def g_back(t, c):
                _, oz_sb, zh_sb = g_state[c]
                h_out = y[c][:, t * CW : (t + 1) * CW]
                n_sb = run.tile([H, CW], BF16, tag=f"n{c}",
                                name=f"n{t}_{c}_{rep}")
                nc.scalar.activation(n_sb[:], pxn[c][:], AF.Tanh)
                nz_sb = run.tile([H, CW], BF16, tag=f"nz{c}",
                                 name=f"nz{t}_{c}_{rep}")
                nc.vector.tensor_mul(nz_sb[:], n_sb[:], oz_sb[:])
                nc.vector.tensor_add(h_out, nz_sb[:], zh_sb[:])

            emit_h0(0)
            emit_h0(1)
            if 0 not in O_tiles:
                load_O(0)
                load_O(1)
            g_in_mms(0, 0)
            for t in range(T + 1):
                if t + 2 < T:
                    load_O(t + 2)
                if t < T:
                    g_rec_mms(t, 0)
                    g_front(t, 0)
                if t >= 1:
                    g_back(t - 1, 1)
                if t < T:
                    g_in_mms(t, 1)
                    g_ident(t, 0)
                    g_rec_mms(t, 1)
                    g_front(t, 1)
                    g_back(t, 0)
                    if t + 1 < T:
                        g_in_mms(t + 1, 0)
                    g_ident(t, 1)

        # The LSTM pool's tiles alias the GRU pool's physical PSUM banks
        # with no dependency edges between pools — hard-barrier between
        # phases so the bank reuse cannot race.
        tc.strict_bb_all_engine_barrier()

        # ================= LSTM phase (own PSUM pool) =================
        with tc.tile_pool(name=f"psumL{rep}", bufs=1, space="PSUM") as ppl:
            pifo = [ppl.tile([128, 3 * CW], F32, name=f"pifo{c}_{rep}")
                    for c in range(NCH)]
            pg = [ppl.tile([128, CW], F32, name=f"pg{c}_{rep}")
                  for c in range(NCH)]

            def l_in_mms(t, c):
                y_t = y[c][:, t * CW : (t + 1) * CW]
                for gi in range(3):
                    gs = slice(gi * E, (gi + 1) * E)
                    mm(pifo[c][0:E, gi * CW : (gi + 1) * CW], wihl_sb[:, gs],
                       y_t, start=True, stop=False)
                mm(pg[c][0:E, :], wihl_sb[:, 3 * E : 4 * E], y_t,
                   start=True, stop=False)

            def l_rec_mms(t, c):
                hl_prev = hl_t[c][t % 2]
                mm(pg[c][0:E, :], whhl_sb[:, 3 * E : 4 * E], hl_prev[:],
                   start=False, stop=True)
                for gi in range(3):
                    gs = slice(gi * E, (gi + 1) * E)
                    mm(pifo[c][0:E, gi * CW : (gi + 1) * CW], whhl_sb[:, gs],
                       hl_prev[:], start=False, stop=True)

            def l_sig(t, c):
                ifo_sb = run.tile([E, 3 * CW], BF16, tag=f"ifo{c}",
                                  name=f"ifo{t}_{c}_{rep}")
                nc.scalar.activation(ifo_sb[:], pifo[c][0:E, :], AF.Sigmoid)
                l_state[c] = [ifo_sb, None]

            def l_g_c(t, c):
                ifo_sb = l_state[c][0]
                g_sb = run.tile([E, CW], BF16, tag=f"gg{c}",
                                name=f"g{t}_{c}_{rep}")
                nc.scalar.activation(g_sb[:], pg[c][0:E, :], AF.Tanh)
                cp = run.tile([E, CW], BF16, tag=f"cp{c}",
                              name=f"cp{t}_{c}_{rep}")
                if t == 0:
                    nc.vector.tensor_mul(cp[:], ifo_sb[:, 0:CW], g_sb[:])
                else:
                    m1_sb = run.tile([E, CW], BF16, tag=f"m1{c}",
                                     name=f"m1{t}_{c}_{rep}")
                    nc.vector.tensor_mul(m1_sb[:], ifo_sb[:, CW : 2 * CW],
                                         c_prev[c][:])
                    m2_sb = run.tile([E, CW], BF16, tag=f"m2{c}",
                                     name=f"m2{t}_{c}_{rep}")
                    nc.vector.tensor_mul(m2_sb[:], ifo_sb[:, 0:CW], g_sb[:])
                    nc.vector.tensor_add(cp[:], m1_sb[:], m2_sb[:])
                c_prev[c] = cp
                l_state[c][1] = cp

            def l_back(t, c):
                ifo_sb, cp = l_state[c]
                hl_new = hl_t[c][(t + 1) % 2]
                tc_sb = run.tile([E, CW], BF16, tag=f"tc{c}",
                                 name=f"tc{t}_{c}_{rep}")
                nc.scalar.activation(tc_sb[:], cp[:], AF.Tanh)
                nc.vector.tensor_mul(hl_new[0:E, :], ifo_sb[:, 2 * CW :],
                                     tc_sb[:])
                # output projection into the pg bank (free after tanh g read;
                # MUST be emitted before the t+1 g-gate input matmul so it
                # does not land inside that accumulation group), f32 copy +
                # f32 DMA out (the bf16 DMA-out path corrupts even columns)
                pout = pg[c][0:V, :]
                mm(pout, outw_sb[:], hl_new[:], start=True, stop=True)
                out_sb = run.tile([V, CW], F32, tag=f"out{c}",
                                  name=f"out{t}_{c}_{rep}", uniquify=True)
                nc.vector.tensor_copy(out_sb[:], pout)
                nc.sync.dma_start(d["logits"][t][:, chs[c]], out_sb[:])
                if t + 1 < T:
                    # next step's input + recurrent matmuls ride behind the
                    # out-projection in the PE queue
                    l_in_mms(t + 1, c)
                    l_rec_mms(t + 1, c)

            l_in_mms(0, 0)
            l_rec_mms(0, 0)
            for t in range(T + 1):
                if t >= 1:
                    l_back(t - 1, 1)
                if t < T:
                    l_sig(t, 0)
                    l_g_c(t, 0)
                    if t == 0:
                        l_in_mms(0, 1)
                        l_rec_mms(0, 1)
                    l_sig(t, 1)
                    l_g_c(t, 1)
                    l_back(t, 0)

        if rep + 1 < reps:
            O_tiles.clear()
            load_O(0)
            load_O(1)
            # next rep's GRU pool re-aliases these banks
            tc.strict_bb_all_engine_barrier()


def _host_prep(inputs):
    import ml_dtypes
    f32 = np.float32
    bf16 = ml_dtypes.bfloat16
    emb = np.asarray(inputs["emb"], f32)
    gru_wih = np.asarray(inputs["gru_wih"], f32)
    gru_whh = np.asarray(inputs["gru_whh"], f32)
    gru_bih = np.asarray(inputs["gru_bih"], f32)
    gru_bhh = np.asarray(inputs["gru_bhh"], f32)
    lstm_wih = np.asarray(inputs["lstm_wih"], f32)
    lstm_whh = np.asarray(inputs["lstm_whh"], f32)
    lstm_bih = np.asarray(inputs["lstm_bih"], f32)
    lstm_bhh = np.asarray(inputs["lstm_bhh"], f32)
    fc_z_w = np.asarray(inputs["fc_z_w"], f32)
    fc_z_b = np.asarray(inputs["fc_z_b"], f32)

    xg_tab = emb @ gru_wih.T + gru_bih
    xg_tab[:, 0:H] += gru_bhh[0:H]
    xg_tab[:, H : 2 * H] += gru_bhh[H : 2 * H]

    hl_init = np.zeros((E + 1, BC), f32)
    hl_init[E, :] = 1.0

    perm = np.concatenate([np.arange(0, 2 * E), np.arange(3 * E, 4 * E),
                           np.arange(2 * E, 3 * E)])
    wih_l = lstm_wih[perm]
    whh_l = lstm_whh[perm]
    b_l = (lstm_bih + lstm_bhh)[perm]

    wih_lT = np.ascontiguousarray(wih_l.T)
    whh_laug = np.concatenate([whh_l.T, b_l[None, :]], axis=0)
    out_w = np.asarray(inputs["out_w"], f32)
    out_b = np.asarray(inputs["out_b"], f32)
    out_waug = np.concatenate([out_w.T, out_b[None, :]], axis=0)

    c = np.ascontiguousarray
    return {
        "hl_init": c(hl_init.astype(bf16)),
        "xg_tab": c(xg_tab.astype(bf16)),
        "bhh_n": c(gru_bhh[2 * H : 3 * H][:, None].astype(f32)),
        "whh_T": c(gru_whh.T.astype(bf16)),
        "fcw_T": c(fc_z_w.T.astype(f32)),
        "fc_b": c(fc_z_b[:, None].astype(f32)),
        "wih_lT": c(wih_lT.astype(bf16)),
        "whh_laug": c(whh_laug.astype(bf16)),
        "ident": c(np.eye(H, dtype=f32).astype(bf16)),
        "out_waug": c(out_waug.astype(bf16)),
    }


_NC_CACHE = {}


def _build(num_devices=N_CORES, reps=1):
    key = (num_devices, reps)
    if key in _NC_CACHE:
        return _NC_CACHE[key]
    nc = bacc.Bacc("TRN2", target_bir_lowering=False, debug=False,
                   num_devices=num_devices)
    d = {}
    for name, shape, dt_ in [
        ("zT", [L, BC], F32R), ("O", [T, V, BC], BF16),
        ("xg_tab", [V, 3 * H], BF16), ("bhh_n", [H, 1], F32),
        ("whh_T", [H, 3 * H], BF16),
        ("fcw_T", [L, H], F32R), ("fc_b", [H, 1], F32),
        ("wih_lT", [H, 4 * E], BF16), ("whh_laug", [E + 1, 4 * E], BF16),
        ("ident", [H, H], BF16), ("hl_init", [E + 1, BC], BF16),
        ("out_waug", [E + 1, V], BF16),
    ]:
        d[name] = nc.dram_tensor(name, shape, dt_, kind="ExternalInput").ap()
    d["logits"] = nc.dram_tensor("logits", [T, V, BC], F32,
                                 kind="ExternalOutput").ap()
    with tile.TileContext(nc) as tc:
        with ExitStack() as ctx:
            _emit(nc, tc, d, ctx, reps=reps)
    nc.compile()
    _NC_CACHE[key] = nc
    return nc


def build_in_maps(inputs):
    import ml_dtypes
    prep = _host_prep(inputs)
    z = np.asarray(inputs["z"], np.float32)
    x_in = np.asarray(inputs["x_in"])
    zT = np.ascontiguousarray(z.T)                       # (L, B)
    O = (x_in[:, :, None] == np.arange(V)[None, None, :])
    O = np.ascontiguousarray(
        np.transpose(O, (1, 2, 0))).astype(ml_dtypes.bfloat16)  # (T, V, B)
    in_maps = []
    for ci in range(N_CORES):
        bs = slice(ci * BC, (ci + 1) * BC)
        m = dict(prep)
        m["zT"] = np.ascontiguousarray(zT[:, bs])
        m["O"] = np.ascontiguousarray(O[:, :, bs])
        in_maps.append(m)
    return in_maps


def assemble_output(results):
    outs = []
    for ci in range(N_CORES):
        lg = results[ci]["logits"]                       # (T, V, BC)
        outs.append(np.ascontiguousarray(np.transpose(lg, (2, 0, 1))))
    return np.concatenate(outs, axis=0).astype(np.float32)  # (B, T, V)


def kernel(**inputs) -> np.ndarray:
    nc = _build()
    in_maps = build_in_maps(inputs)
    res = run_bass_kernel_spmd(nc, in_maps, list(range(N_CORES)))
    return assemble_output(res.results)
